# revision 22
# baseline (speedup 1.0000x reference)
"""Trainium2 Bass kernel for CARE position encoding (rotor sandwich product).

The reference computes out = R x R~ where R is a product of 4 plane rotors
(cos(phi_i) + sin(phi_i) e_mi) with phi_i = 0.5 * c_i * theta[pos, i].
Algebraically this factorizes into 4 sequential Givens-rotation stages: for
plane bivector e_m, the 8 basis blades A with |A & m| == 1 rotate in 4
disjoint pairs (A, A^m) by angle 2*phi with pair signs tau = C[A, m, A^m];
the other 8 blades pass through unchanged:
    out[a] = c2*x[a] + tau*s2*x[b] ;  out[b] = c2*x[b] - tau*s2*x[a]

Implementation (data-parallel across 8 cores, batch-sharded, 2 rows/core):
 - component-PLANAR fp16 SBUF layout: the host reorders x per core to
   [P, 16 slots, J] where slot pairs (2p, 2p+1) hold dual blades (A, ~A).
   Duality preserves |A & m| parity, so every plane's 8 rotating blades are
   4 whole slot pairs, and a fixed pair->slot assignment makes each plane's
   T (cos part), U (sin part) and ADD a SINGLE strided DVE op whose
   innermost dim is a contiguous, 4B-aligned run of positions -- which lets
   every fp16 tensor_tensor run in the DVE's packed 2x mode.
 - angle handling is branch-per-plane at build time on the compile-time
   max |angle| = 16383*|f_i*c_i|:
     * small (<= ~pi/2): the sin/cos tables are emitted as single ScalarE
       Sin activations DIRECTLY from the int16 pos tile (arg = scale*pos
       [+ pi/2 for cos]) -- no angle tile, no range reduction, no DVE work.
     * large: A = fc*pos and the magic-rounded K = round(A/2pi) on ScalarE,
       then the otherwise-idle TensorEngine computes R = A - 2pi*K as a
       pair of accumulating fp32 matmuls with NEFF-embedded identity /
       -2pi*identity weights, into PSUM (error <= |K|*ulp(2pi) ~ 1e-5 rad,
       far inside the 2e-2 tolerance; keeps the reduction off the
       saturated DVE).  ScalarE reads R straight from PSUM: |R|, then Sin
       emits the fp16 cos block (arg = pi/2 - |R|) and the per-plane
       sign-sequenced sin region.
 - per-engine queue order == emission order, so tables/stages are emitted
   interleaved: stage-1/2 tables (pure ScalarE, no reduction) come first
   and the first TensorTensor starts ~2us after the pos DMA lands; the
   DVE then runs the 4 stages back-to-back with no queue-head stalls.
 - the last stage's in-place ADD is split {4,2,2} slots; each piece's
   output DMA is issued as its ADD lands, alternating the SP/ACT HWDGE
   rings so the tail transfers overlap the remaining adds.  (Finer
   splitting measurably regresses: every extra DMA trigger and tiny DVE
   op costs more in sequencer/semaphore overhead than it overlaps.)
 - every plane's index arithmetic is verified symbolically against the
   input Cayley tensor at kernel-build time.
"""
import numpy as np

import concourse.bass as bass
import concourse.tile as tile
from concourse import bacc, mybir
from concourse.bass_utils import run_bass_kernel_spmd

F32 = mybir.dt.float32
F16 = mybir.dt.float16
I32 = mybir.dt.int32
I16 = mybir.dt.int16

P = 128
NCORES = 8
B, L, MV = 16, 16384, 16
MAX_LEN = 16384
ROWS_PER_CORE = B // NCORES          # 2
N = ROWS_PER_CORE * L                # 32768 positions per core
J = N // P                           # 256 positions per partition

PLANE_BLADES = (3, 5, 9, 6)
STAGE_ORDER = (6, 9, 5, 3)           # sandwich applies the last rotor first

# Dual slot pairing: pair p holds blades (first, second) = (A, 15^A) at
# slot planes (2p, 2p+1).  Pair classes (by which planes rotate them) and
# the within-pair order are chosen so each plane's rotating pairs and
# their XOR-partner map are affine in <=2 AP dims (see _PLANE_OPS).
PAIRS = ((6, 9), (2, 13), (1, 14), (5, 10), (3, 12), (8, 7), (4, 11), (0, 15))
COMP_OF_SLOT = tuple(c for pr in PAIRS for c in pr)
SLOT_OF_COMP = tuple(COMP_OF_SLOT.index(c) for c in range(16))

MAGIC = float(np.float32(1.5 * 2 ** 23))
TWO_PI = 2.0 * np.pi
INV_2PI = float(np.float32(1.0 / TWO_PI))
HALF_PI = float(np.float32(np.pi / 2.0))
TWO_PI_F = float(np.float32(TWO_PI))

# compile-time |angle| bound below which a plane needs no range reduction
# (cos arg pi/2 - A stays in Sin's [-pi, pi] domain without an Abs)
SMALL_ANGLE = 1.5

# planes whose every T-sub has <=2 slot dims read a non-duplicated [P, J]
# cos table via a stride-0 pair dim (3 free AP dims); m=3's T needs 3 slot
# dims so its table keeps the duplicated [c,c] layout (4 free dims is over
# the AP limit)
NONDUP_CD = (6, 9, 5)

# per-plane SSX block sign sequences (block b holds PLANE_SEQ[m][b] * sin)
_PLANE_SEQ = {3: (1, -1, 1), 5: (1, 1, -1, -1), 9: (1, -1, 1), 6: (1, -1, 1)}

# Per-plane op descriptors in SLOT-PLANE units (strides/offsets are
# multiples of J elements).  Each sub:
#   x0/xd : slot-plane offset / [step,count] dims of the X read (T,U) or
#           X write (ADD, which reuses the T sub's x side)
#   t0/td : offset / dims in the T/U tile (8 slot planes, pair-major)
#   s0/sd (U only): SSX block offset / per-dim block steps
# The AP builder appends the position dim [1, J]; when every operand's
# innermost slot dim is [1, 2] (adjacent slot planes / adjacent blocks) it
# is merged with the position dim into a contiguous [1, 2J] run.
_PLANE_OPS = {
    3: dict(  # pairs (G,B,A,F) = slots 0..7, partners reversed.  T/U are
        # single whole-stage ops; only the in-place ADD is split in two
        # 4-slot halves so each half's output DMA starts as soon as its
        # ADD lands.
        t=[dict(x0=0, xd=[[4, 2], [2, 2], [1, 2]],
                t0=0, td=[[4, 2], [2, 2], [1, 2]])],
        u=[dict(x0=6, xd=[[-4, 2], [-2, 2], [1, 2]],
                t0=0, td=[[4, 2], [2, 2], [1, 2]],
                s0=1, sd=[[-1, 2], [0, 2], [1, 2]])],
        a=[dict(x0=0, xd=[[2, 2], [1, 2]], t0=0, td=[[2, 2], [1, 2]]),
           dict(x0=4, xd=[[1, 2]], t0=4, td=[[1, 2]]),
           dict(x0=6, xd=[[1, 2]], t0=6, td=[[1, 2]])],
    ),
    5: dict(  # pairs (G,A,E,C) = slots {0,2,4,6}; partners = +4 mod 8
        t=[dict(x0=0, xd=[[4, 4], [1, 2]],
                t0=0, td=[[2, 4], [1, 2]])],
        u=[dict(x0=8, xd=[[-8, 2], [4, 2], [1, 2]],
                t0=0, td=[[4, 2], [2, 2], [1, 2]],
                s0=0, sd=[[2, 2], [0, 2], [1, 2]])],
    ),
    9: dict(  # pairs (A,F,E,D) = slots 2..5; A<->D keep order, F<->E flip
        t=[dict(x0=4, xd=[[2, 4], [1, 2]],
                t0=0, td=[[2, 4], [1, 2]])],
        u=[dict(x0=10, xd=[[-6, 2], [1, 2]],
                t0=0, td=[[6, 2], [1, 2]],
                s0=0, sd=[[1, 2], [1, 2]]),
           dict(x0=9, xd=[[-2, 2], [-1, 2]],
                t0=2, td=[[2, 2], [1, 2]],
                s0=1, sd=[[0, 2], [-1, 2]])],
    ),
    6: dict(  # pairs (B,F,E,C); split into the {F,E} half (slots 6..9,
        # covered by the first x chunk, so it starts before slots 2,3,12,13
        # land) and the {B,C} half (slots 2,3,12,13)
        t=[dict(x0=6, xd=[[2, 2], [1, 2]], t0=2, td=[[2, 2], [1, 2]]),
           dict(x0=2, xd=[[10, 2], [1, 2]], t0=0, td=[[6, 2], [1, 2]])],
        u=[dict(x0=8, xd=[[-2, 2], [1, 2]], t0=2, td=[[2, 2], [1, 2]],
                s0=1, sd=[[-1, 2], [1, 2]]),
           dict(x0=12, xd=[[-10, 2], [1, 2]], t0=0, td=[[6, 2], [1, 2]],
                s0=0, sd=[[1, 2], [1, 2]])],
    ),
}


def _iter_idx(dims):
    import itertools
    return itertools.product(*[range(c) for (_, c) in dims])


def _off(dims, idx):
    return sum(s * i for (s, _), i in zip(dims, idx))


def _verify_plane_ops(cayley):
    """Symbolically apply the descriptor index arithmetic for one position:
    out[comp] = c2*x[tcomp] + seqsign*s2*x[ucomp], and check it equals the
    Cayley-derived Givens stage for every plane.  Raises on mismatch."""
    for m in PLANE_BLADES:
        ops = _PLANE_OPS[m]
        tmap, umap, usgn, amap = {}, {}, {}, {}
        for sub in ops["t"]:
            for idx in _iter_idx(sub["xd"]):
                tp = sub["t0"] + _off(sub["td"], idx)
                sp = sub["x0"] + _off(sub["xd"], idx)
                assert tp not in tmap, (m, tp)
                tmap[tp] = sp
        for sub in ops.get("a", ops["t"]):
            for idx in _iter_idx(sub["xd"]):
                tp = sub["t0"] + _off(sub["td"], idx)
                sp = sub["x0"] + _off(sub["xd"], idx)
                assert tp not in amap, (m, tp)
                amap[tp] = sp
        assert amap == tmap, (m, amap, tmap)
        seq = _PLANE_SEQ[m]
        for sub in ops["u"]:
            for idx in _iter_idx(sub["xd"]):
                tp = sub["t0"] + _off(sub["td"], idx)
                sp = sub["x0"] + _off(sub["xd"], idx)
                blk = sub["s0"] + _off(list(zip([s for s, _ in sub["sd"]],
                                                [c for _, c in sub["xd"]])), idx)
                assert 0 <= blk < len(seq), (m, blk)
                assert tp not in umap, (m, tp)
                umap[tp] = sp
                usgn[tp] = seq[blk]
        assert sorted(tmap) == sorted(umap) == list(range(8)), m
        for tp in range(8):
            a = COMP_OF_SLOT[tmap[tp]]
            b = a ^ m
            assert COMP_OF_SLOT[umap[tp]] == b, (m, tp, COMP_OF_SLOT[umap[tp]], b)
            tau = float(cayley[a, m, b])
            assert usgn[tp] == tau, (m, tp, usgn[tp], tau)
        # every rotating slot pair must be read/written exactly once
        rot = sorted(tmap.values())
        expect = sorted(s for s in range(16)
                        if bin(COMP_OF_SLOT[s] & m).count("1") == 1)
        assert rot == expect, (m, rot, expect)


def _ap_with_dims(base_ap, extra_off, dims):
    ap = [list(base_ap.ap[0])] + [list(d) for d in dims]
    return bass.AP(base_ap.tensor, base_ap.offset + extra_off, ap)


def _merged(sub):
    md = all(d[-1] == [1, 2] for d in (sub["xd"], sub["td"]))
    if "sd" in sub:
        md = md and sub["sd"][-1][0] == 1
    return md


def _el(slotdims, merged):
    if merged:
        return [[s * J, n] for s, n in slotdims[:-1]] + [[1, 2 * J]]
    return [[s * J, n] for s, n in slotdims] + [[1, J]]


def _el_blk(blksteps, counts, merged):
    if merged:
        return [[s * J, n] for (s, _), (_, n) in
                zip(blksteps[:-1], counts[:-1])] + [[1, 2 * J]]
    return [[s * J, n] for (s, _), (_, n) in zip(blksteps, counts)] + [[1, J]]


def _cdims(sub, merged):
    if merged:
        return [[0, n] for _, n in sub["td"][:-1]] + [[1, 2 * J]]
    return [[0, n] for _, n in sub["td"]] + [[1, J]]


def _build_program(freqs, coefs):
    nc = bacc.Bacc("TRN2", target_bir_lowering=False, debug=False,
                   enable_asserts=False, num_devices=NCORES)
    x_d = nc.dram_tensor("x", [P, 16 * J], F16, kind="ExternalInput")
    pos_d = nc.dram_tensor("pos", [P, J], I16, kind="ExternalInput")
    out_d = nc.dram_tensor("out", [P, 16 * J], F16, kind="ExternalOutput")
    # NEFF-embedded constants for the PE-side range reduction (loaded to
    # HBM at model-load time, not during execution)
    id_d = nc.inline_tensor(np.eye(P, dtype=np.float32), name="id128")
    n2p_d = nc.inline_tensor(
        (-np.float32(TWO_PI_F) * np.eye(P)).astype(np.float32), name="n2pi128")

    SIN = mybir.ActivationFunctionType.Sin
    COPY = mybir.ActivationFunctionType.Copy
    IDENT = mybir.ActivationFunctionType.Identity
    ABS = mybir.ActivationFunctionType.Abs
    plane_i = {m: PLANE_BLADES.index(m) for m in STAGE_ORDER}

    with tile.TileContext(nc) as tc:
        with tc.tile_pool(name="const", bufs=1) as cpool, \
             tc.tile_pool(name="ang", bufs=2) as apool, \
             tc.tile_pool(name="tmp", bufs=2) as tpool, \
             tc.psum_pool(name="pp", bufs=2) as ppool:

            X = cpool.tile([P, 16 * J], F16)
            Pp = cpool.tile([P, J], I16)
            # pos first (it gates the whole table pipeline), then X in
            # stage-need order: m=6 {F,E} half (slots 6-9), its {B,C} half
            # (slots 2,3 / 12,13), then the rest.  All on the SP HWDGE ring:
            # FIFO order = emission order, so pos drains at full rate first.
            nc.sync.dma_start(Pp[:], pos_d[:])
            nc.sync.dma_start(X[:, 6 * J:10 * J], x_d[:, 6 * J:10 * J])
            nc.sync.dma_start(X[:, 2 * J:4 * J], x_d[:, 2 * J:4 * J])
            nc.sync.dma_start(X[:, 12 * J:14 * J], x_d[:, 12 * J:14 * J])
            nc.sync.dma_start(X[:, 4 * J:6 * J], x_d[:, 4 * J:6 * J])
            nc.sync.dma_start(X[:, 10 * J:12 * J], x_d[:, 10 * J:12 * J])
            nc.sync.dma_start(X[:, :2 * J], x_d[:, :2 * J])
            IDs = cpool.tile([P, P], F32, name="ids")
            N2Ps = cpool.tile([P, P], F32, name="n2ps")
            nc.sync.dma_start(IDs[:], id_d[:])
            nc.sync.dma_start(N2Ps[:], n2p_d[:])
            # slots 14,15 (pair (0,15)) are never rotated: DRAM->DRAM pass
            nc.sync.dma_start(out_d[:, 14 * J:], x_d[:, 14 * J:])

            CB = {}
            for ci, v in enumerate((HALF_PI, MAGIC, -MAGIC)):
                cb = cpool.tile([P, 1], F32, name=f"bias{ci}")
                nc.gpsimd.memset(cb[:], v)
                CB[v] = cb
            # dummy Sin with no pos dependency as the FIRST ScalarE op: the
            # auto-inserted ACT_TABLE_LOAD lands in front of it and runs at
            # kernel start.  Without it the load sits behind the scheduler's
            # pos-DMA wait and its 1.3us moves onto the critical path.
            warm = cpool.tile([P, 1], F32, name="warm")
            nc.scalar.activation(warm[:], CB[HALF_PI][:], SIN)

            # ---- per-plane sin/cos tables + Givens stages, interleaved ----
            # Emission order IS per-engine queue order, so it is arranged
            # so that (a) the ScalarE queue runs stage-1/2 tables first and
            # the big planes' pos-only prep (A, K1, K) right behind them,
            # (b) the two DVE range-reduction ops (R = A - 2pi*K) sit
            # BETWEEN stage TT groups (emitting them earlier would park
            # them at the head of the in-order DVE queue, stalling every
            # TensorTensor behind a ScalarE dependency), and (c) each
            # plane's Abs/Sin block goes on ScalarE as soon as its R edge
            # is in the DVE queue.
            Cd, SX = {}, {}
            ang = {}  # m -> dict(A, K, R, RC) for large-angle planes
            fcs, smalls = {}, {}
            for m in STAGE_ORDER:
                i = plane_i[m]
                fc = float(np.float32(np.float32(freqs[i]) *
                                      np.float32(coefs[i])))
                fcs[m] = fc
                smalls[m] = MAX_LEN * abs(fc) <= SMALL_ANGLE
                cdw = J if m in NONDUP_CD else 2 * J
                Cd[m] = cpool.tile([P, cdw], F16, name=f"cd{m}")
                SX[m] = cpool.tile([P, len(_PLANE_SEQ[m]) * J], F16,
                                   name=f"sx{m}")

            def emit_sin_blocks(m, src, scale_base):
                seq = _PLANE_SEQ[m]
                for sgn in (1.0, -1.0):
                    blks = [b for b, s in enumerate(seq) if s == sgn]
                    if len(blks) == 1:
                        od = [[1, J]]
                    else:
                        od = [[(blks[1] - blks[0]) * J, len(blks)], [1, J]]
                    nc.scalar.activation(
                        _ap_with_dims(SX[m][:], blks[0] * J, od),
                        _ap_with_dims(src, 0,
                                      [[0, len(blks)], [1, J]][-len(od):]),
                        SIN,
                        scale=float(np.float32(sgn) * np.float32(scale_base)))

            def emit_cd(m, src_ap, scale):
                if m in NONDUP_CD:
                    nc.scalar.activation(Cd[m][:], src_ap, SIN,
                                         scale=scale, bias=CB[HALF_PI][:])
                else:
                    nc.scalar.activation(
                        _ap_with_dims(Cd[m][:], 0, [[J, 2], [1, J]]),
                        _ap_with_dims(src_ap, 0, [[0, 2], [1, J]]), SIN,
                        scale=scale, bias=CB[HALF_PI][:])

            def emit_small_tables(m):
                # straight from pos: sin(fc*pos), cos = sin(pi/2 - fc*pos);
                # |fc*pos| <= SMALL_ANGLE keeps both inside Sin's domain
                emit_cd(m, Pp[:], -fcs[m])
                emit_sin_blocks(m, Pp[:], fcs[m])

            # Per big plane, one pair of accumulating PE matmuls computes
            # R = I.T@A + (-2pi*I).T@K into PSUM.  This keeps the range
            # reduction entirely off the saturated DVE; ScalarE reads R
            # straight from PSUM.  Per-plane (not combined) so R for the
            # first big plane is ready right after ITS K, not the slower
            # plane's.  (A ScalarE-written PSUM bank + start=False
            # accumulate passes CoreSim but yields NaN on hardware -- the
            # accumulation group must be matmul-initialized.)  No
            # Cody-Waite: the residual |K|*ulp(2pi) ~ 1e-5 rad is
            # negligible here.
            def emit_big_prep(m):
                A = apool.tile([P, J], F32, tag="a")
                K1 = apool.tile([P, J], F32, tag="k1")
                K = apool.tile([P, J], F32, tag="k")
                RC = apool.tile([P, J], F32, tag="rc")
                Rp = ppool.tile([P, J], F32, tag="rp")
                nc.scalar.activation(A[:], Pp[:], COPY, scale=fcs[m])
                nc.scalar.activation(K1[:], Pp[:], IDENT,
                                     bias=CB[MAGIC][:],
                                     scale=float(np.float32(fcs[m]) *
                                                 INV_2PI))
                nc.scalar.activation(K[:], K1[:], IDENT, bias=CB[-MAGIC][:])
                ang[m] = dict(A=A, K=K, RC=RC, Rp=Rp)

            def emit_big_reduce(m):
                a = ang[m]
                nc.tensor.matmul(a["Rp"][:], IDs[:], a["A"][:],
                                 start=True, stop=False)
                nc.tensor.matmul(a["Rp"][:], N2Ps[:], a["K"][:],
                                 start=False, stop=True)

            def emit_big_tables(m):
                a = ang[m]
                R = a["Rp"][:]
                nc.scalar.activation(a["RC"][:], R, ABS)
                emit_cd(m, a["RC"][:], -1.0)
                emit_sin_blocks(m, R, 1.0)

            def emit_stage(si, m):
                ops = _PLANE_OPS[m]
                T = tpool.tile([P, 8 * J], F16, tag="t")
                U = tpool.tile([P, 8 * J], F16, tag="u")

                def emit_t(sub):
                    if m in NONDUP_CD:
                        # non-dup [P, J] cos table: stride-0 pair dim
                        tdims = [[s * J, n] for s, n in sub["td"][:-1]] +                             [[J, 2], [1, J]]
                        xdims = [[s * J, n] for s, n in sub["xd"][:-1]] +                             [[J, 2], [1, J]]
                        cdims = [[0, n] for _, n in sub["td"][:-1]] +                             [[0, 2], [1, J]]
                        nc.vector.tensor_mul(
                            _ap_with_dims(T[:], sub["t0"] * J, tdims),
                            _ap_with_dims(X[:], sub["x0"] * J, xdims),
                            _ap_with_dims(Cd[m][:], 0, cdims))
                        return
                    md = _merged(sub)
                    nc.vector.tensor_mul(
                        _ap_with_dims(T[:], sub["t0"] * J, _el(sub["td"], md)),
                        _ap_with_dims(X[:], sub["x0"] * J, _el(sub["xd"], md)),
                        _ap_with_dims(Cd[m][:], 0, _cdims(sub, md)))

                def emit_u(sub):
                    md = _merged(sub)
                    nc.vector.tensor_mul(
                        _ap_with_dims(U[:], sub["t0"] * J, _el(sub["td"], md)),
                        _ap_with_dims(X[:], sub["x0"] * J, _el(sub["xd"], md)),
                        _ap_with_dims(SX[m][:], sub["s0"] * J,
                                      _el_blk(sub["sd"], sub["xd"], md)))

                def emit_a(sub):
                    md = _merged(sub)
                    nc.vector.tensor_add(
                        _ap_with_dims(X[:], sub["x0"] * J, _el(sub["xd"], md)),
                        _ap_with_dims(T[:], sub["t0"] * J, _el(sub["td"], md)),
                        _ap_with_dims(U[:], sub["t0"] * J, _el(sub["td"], md)))

                if si == 0:
                    # the {F,E} half's X chunk lands first: run its whole
                    # T/U/ADD before the {B,C} half's chunks are needed
                    # (ADD writes slots 6-9, disjoint from {B,C} reads)
                    for k in range(len(ops["t"])):
                        emit_t(ops["t"][k])
                        emit_u(ops["u"][k])
                        emit_a(ops["t"][k])
                    return
                for sub in ops["t"]:
                    emit_t(sub)
                for sub in ops["u"]:
                    emit_u(sub)
                asubs = ops.get("a", ops["t"])
                for ai, sub in enumerate(asubs):
                    emit_a(sub)
                    if si == 3:
                        # each final-stage piece DMAs out as its ADD lands,
                        # alternating HWDGE rings so tail transfers overlap
                        if ai == 0:
                            nc.sync.dma_start(out_d[:, :4 * J], X[:, :4 * J])
                        elif ai == 1:
                            nc.scalar.dma_start(out_d[:, 4 * J:6 * J],
                                                X[:, 4 * J:6 * J])
                        else:
                            nc.sync.dma_start(out_d[:, 6 * J:8 * J],
                                              X[:, 6 * J:8 * J])
                if si == 1:
                    # pair D (slots 10,11) is final after stage 2 (m=9)
                    nc.sync.dma_start(out_d[:, 10 * J:12 * J],
                                      X[:, 10 * J:12 * J])
                if si == 2:
                    # pairs E,C (slots 8,9 / 12,13) final after stage 3
                    nc.sync.dma_start(out_d[:, 8 * J:10 * J],
                                      X[:, 8 * J:10 * J])
                    nc.sync.dma_start(out_d[:, 12 * J:14 * J],
                                      X[:, 12 * J:14 * J])

            m1, m2, m3_, m4 = STAGE_ORDER
            if smalls[m1] and smalls[m2] and not smalls[m3_] \
                    and not smalls[m4]:
                emit_small_tables(m1)
                emit_big_prep(m3_)
                emit_small_tables(m2)
                emit_big_reduce(m3_)  # PE; fires as soon as K(m3_) lands
                emit_big_prep(m4)
                emit_big_reduce(m4)
                emit_stage(0, m1)
                emit_big_tables(m3_)
                emit_stage(1, m2)
                emit_big_tables(m4)
                emit_stage(2, m3_)
                emit_stage(3, m4)
            else:
                # generic (slower) order for an unexpected angle pattern
                for si, m in enumerate(STAGE_ORDER):
                    if smalls[m]:
                        emit_small_tables(m)
                    else:
                        emit_big_prep(m)
                        emit_big_reduce(m)
                for m in STAGE_ORDER:
                    if not smalls[m]:
                        emit_big_tables(m)
                for si, m in enumerate(STAGE_ORDER):
                    emit_stage(si, m)

    nc.compile()
    return nc


_PROGRAM_CACHE = {}


def _get_program(freqs, coefs):
    key = (tuple(freqs), tuple(coefs))
    if key not in _PROGRAM_CACHE:
        _PROGRAM_CACHE[key] = _build_program(freqs, coefs)
    return _PROGRAM_CACHE[key]


def _derive_params(inputs):
    coefs = [float(np.asarray(inputs[c], dtype=np.float32).reshape(MV)[b])
             for c, b in zip(("bx", "by", "bz", "bw"), PLANE_BLADES)]
    theta = np.asarray(inputs["theta"], dtype=np.float32)
    freqs = [float(theta.reshape(MAX_LEN, 4)[1, i]) for i in range(4)]
    return freqs, coefs


def _core_input(x, pos, g):
    xg = np.asarray(x[g * ROWS_PER_CORE:(g + 1) * ROWS_PER_CORE],
                    dtype=np.float32).reshape(P, J, MV)
    planar = xg[:, :, COMP_OF_SLOT].transpose(0, 2, 1)
    pg = np.clip(pos[g * ROWS_PER_CORE:(g + 1) * ROWS_PER_CORE],
                 0, MAX_LEN - 1).astype(np.int16).reshape(P, J)
    return {"x": np.ascontiguousarray(planar.astype(np.float16)
                                      ).reshape(P, 16 * J),
            "pos": np.ascontiguousarray(pg)}


def _core_output(res_g):
    r = np.asarray(res_g).reshape(P, 16, J).transpose(0, 2, 1)
    return r[:, :, SLOT_OF_COMP].astype(np.float32).reshape(
        ROWS_PER_CORE, L, MV)


def kernel(x, pos, bx, by, bz, bw, theta, cayley, biv_mask, scalar_mask):
    x = np.asarray(x, dtype=np.float32)
    pos = np.asarray(pos)
    theta = np.asarray(theta, dtype=np.float32)
    cayley = np.asarray(cayley, dtype=np.float32)

    assert x.shape == (B, L, MV) and pos.shape == (B, L)

    freqs, coefs = _derive_params(
        dict(bx=bx, by=by, bz=bz, bw=bw, theta=theta))
    th_check = np.arange(MAX_LEN, dtype=np.float32)[:, None] * \
        np.asarray(freqs, dtype=np.float32)[None, :]
    assert np.array_equal(th_check, theta.reshape(MAX_LEN, 4)), \
        "theta table is not linear in position; kernel assumption violated"

    _verify_plane_ops(cayley)

    nc = _get_program(freqs, coefs)

    in_maps = [_core_input(x, pos, g) for g in range(NCORES)]
    res = run_bass_kernel_spmd(nc, in_maps, core_ids=list(range(NCORES)))
    out = np.empty((B, L, MV), dtype=np.float32)
    for g in range(NCORES):
        out[g * ROWS_PER_CORE:(g + 1) * ROWS_PER_CORE] = \
            _core_output(res.results[g]["out"])
    return out
